# revision 23
# baseline (speedup 1.0000x reference)
"""CapsuleLayer (dynamic routing) Trainium2 Bass kernel — v2.

Math (per example b):
  u_hat[b,i,o,n] = sum_v x[b,i,v] * W[i,o,v,n]        I=1152, O=10, V=8, N=16
  b_logits = 0; repeat n_routing times:
    c = softmax_o(b_logits); s = sum_i c*u_hat; out = squash(s)
    if not last: b_logits += sum_n u_hat*out

Distribution: batch B=256 sharded over 8 cores (32 each). W replicated.

Per-core layout (chunk = 8 examples, 4 chunks), i = ib*16 + il:
  K partitions k = il*8+v   (contraction rows of the u_hat matmul)
  M partitions p = b*16+il  (rows of u_hat / routing state)
  U[c] [128, 72, 160] bf16  u_hat,  U[(b,il), ib, (o,n)]
  XBD  [128, 9, 128] bf16 slabs of block-diag x: XBD[(il,v),ib,(b,il')]
  CBD[c] [128, 80, 72] bf16 block-diag c: CBD[(b,il),(o,b'),ib]
  w2   [128, 72, 160] bf16  W2[(il,v), ib, (o,n)] = W[ib*16+il, o, v, n]
  u_hat matmul (per ib): psum[(b,il'),(o,n)] = XBD[:,ib,:].T @ w2[:,ib,:]
  s matmul (per iter): psum[(o,b),(o',n)] += CBD[:,:,ib].T @ U[:,ib,:]

v2 structure (vs v1 pairs): all 4 chunks in flight; iters outer, chunks
inner, with phase-1 of chunks 2/3 interleaved after routing of chunks 0/1
starts, so PE runs continuously and DVE agreement pipelines across chunks.
Diagonal s-extraction is mask-multiply + tiny selection matmuls (replacing
10 small DMAs per chunk-iter); a second selection matmul replicates s to
all 128 partitions for the agreement product. Small DMAs (cbd scatter,
output) ride the gpsimd queue (cheap issue). psum->U casts round-robin
over DVE/ACT/GPSIMD. Routing-side tensors (b-logits, softmax, agreement
tree) are bf16 (DVE 2x); squash runs fp32 with ACT doing the pointwise
chain and a DVE-only rsqrt (bit hack + 2 Newton steps).
"""

import os
import sys

import numpy as np

_TRN_REPO = "/opt/trn_rl_repo"
if _TRN_REPO not in sys.path:
    sys.path.insert(0, _TRN_REPO)

EPS = 1e-10
B, I, V, O, N = 256, 1152, 8, 10, 16
NCORES = 8
BLOC = B // NCORES          # 32 examples per core
BC = 8                      # examples per chunk
NCHUNK = BLOC // BC         # 4
IB = I // 16                # 72 i-blocks
ON = O * N                  # 160
NSLAB = 8                   # phase-1 xbd slabs per chunk
SLAB = IB // NSLAB          # 9 ibs per slab
RSQRT_MAGIC = 0x5F3759DF


def _build(n_routing: int):
    import concourse.bacc as bacc
    import concourse.tile as tile
    from concourse import mybir

    nc = bacc.Bacc("TRN2", target_bir_lowering=False, debug=False)
    f32 = mybir.dt.float32
    bf16 = mybir.dt.bfloat16

    xbdh = nc.dram_tensor(
        "xbdh", [NCHUNK, 128, IB, 128], bf16, kind="ExternalInput"
    )
    w2 = nc.dram_tensor("w2", [128, IB, ON], bf16, kind="ExternalInput")
    e2 = nc.dram_tensor("e2", [128, 80], bf16, kind="ExternalInput")
    dmask = nc.dram_tensor("dmask", [80, ON], bf16, kind="ExternalInput")
    selb = nc.dram_tensor("selb", [80, BC], bf16, kind="ExternalInput")
    selr = nc.dram_tensor("selr", [80, 128], bf16, kind="ExternalInput")
    out_d = nc.dram_tensor("out", [BLOC, O, N], f32, kind="ExternalOutput")

    with tile.TileContext(nc) as tc:
        with (
            tc.tile_pool(name="state", bufs=1) as state,
            tc.tile_pool(name="p1", bufs=1) as p1,
            tc.tile_pool(name="small", bufs=2) as small,
            tc.tile_pool(name="tree", bufs=1) as tree,
            tc.tile_pool(name="psA", bufs=3, space="PSUM") as psA,
            tc.tile_pool(name="psS", bufs=3, space="PSUM") as psS,
            tc.tile_pool(name="psE", bufs=1, space="PSUM") as psE,
            tc.tile_pool(name="psX", bufs=1, space="PSUM") as psX,
        ):
            Us = [
                state.tile([128, IB, ON], bf16, tag=f"U{j}", name=f"U{j}")
                for j in range(NCHUNK)
            ]
            cbds = [
                state.tile([128, 80, IB], bf16, tag=f"cbd{j}", name=f"cbd{j}")
                for j in range(NCHUNK)
            ] if n_routing > 1 else []
            bbs = [
                state.tile([128, IB, O], bf16, tag=f"bb{j}", name=f"bb{j}")
                for j in range(NCHUNK)
            ]
            e2s = state.tile([128, 80], bf16)
            nc.sync.dma_start(out=e2s[:], in_=e2[:])
            dmask_s = state.tile([80, ON], bf16)
            nc.sync.dma_start(out=dmask_s[:], in_=dmask[:])
            selb_s = state.tile([80, BC], bf16)
            nc.sync.dma_start(out=selb_s[:], in_=selb[:])
            selr_s = state.tile([80, 128], bf16)
            nc.sync.dma_start(out=selr_s[:], in_=selr[:])
            w2s = state.tile([128, IB, ON], bf16)
            for sl in range(NSLAB):
                nc.sync.dma_start(
                    out=w2s[:, sl * SLAB:(sl + 1) * SLAB, :],
                    in_=w2[:, sl * SLAB:(sl + 1) * SLAB, :],
                )
            # zero block-diag c while phase 1 runs (gpsimd is idle)
            for cb in cbds:
                nc.gpsimd.memset(cb[:], 0.0)
            scb = state.tile([32, O], bf16)
            nc.gpsimd.memset(scb[:], 0.0)

            def phase1(c):
                for sl in range(NSLAB):
                    xbd = p1.tile([128, SLAB, 128], bf16, tag="xbd", bufs=2)
                    nc.gpsimd.dma_start(
                        out=xbd[:], in_=xbdh[c, :, sl * SLAB:(sl + 1) * SLAB, :]
                    )
                    for g in range(SLAB // 3):
                        ps = psA.tile([128, 3, ON], f32, tag="psA")
                        for j in range(3):
                            ib = g * 3 + j
                            nc.tensor.matmul(
                                ps[:, j, :],
                                xbd[:, ib, :],
                                w2s[:, sl * SLAB + ib, :],
                                start=True,
                                stop=True,
                            )
                        dst = Us[c][:, sl * SLAB + g * 3:sl * SLAB + (g + 1) * 3, :]
                        if (sl * 3 + g) % 3 == 0:
                            nc.vector.tensor_copy(dst, ps[:])
                        else:
                            nc.scalar.copy(dst, ps[:])

            f32 = mybir.dt.float32

            def s_part(c, it):
                pss = psS.tile([80, ON], f32, tag="psS")
                for ib in range(IB):
                    lhsT = e2s[:] if it == 0 else cbds[c][:, :, ib]
                    nc.tensor.matmul(
                        pss[:], lhsT, Us[c][:, ib, :],
                        start=(ib == 0), stop=(ib == IB - 1),
                    )
                return pss

            def rest(c, it, pss):
                _routing_rest(
                    nc, tc, mybir, small, tree, psE, psX, pss,
                    Us[c], cbds[c] if cbds else None, bbs[c], scb,
                    dmask_s, selb_s, selr_s, out_d, c, it, n_routing,
                )

            # interleave phase 1 with the first routing iteration, and emit
            # each chunk's extraction after the next chunk's s-matmuls so PE
            # never waits on the (DVE) mask step
            phase1(0)
            phase1(1)
            ps0 = s_part(0, 0)
            phase1(2)
            ps1 = s_part(1, 0)
            rest(0, 0, ps0)
            phase1(3)
            ps2 = s_part(2, 0)
            rest(1, 0, ps1)
            ps3 = s_part(3, 0)
            rest(2, 0, ps2)
            rest(3, 0, ps3)
            for it in range(1, n_routing):
                ps0 = s_part(0, it)
                ps1 = s_part(1, it)
                rest(0, it, ps0)
                ps2 = s_part(2, it)
                rest(1, it, ps1)
                ps3 = s_part(3, it)
                rest(2, it, ps2)
                rest(3, it, ps3)

    nc.compile()
    return nc


def _routing_rest(nc, tc, mybir, small, tree, psE, psX, pss, U, cbd, bb,
                  scb, dmask_s, selb_s, selr_s, out_d, c, it, n_routing):
    f32 = mybir.dt.float32
    bf16 = mybir.dt.bfloat16
    i32 = mybir.dt.int32
    AX = mybir.AxisListType
    OP = mybir.AluOpType
    AF = mybir.ActivationFunctionType
    last = it == n_routing - 1

    # mask off-diagonal o-blocks, then sum partition o-groups with tiny
    # matmuls: s3p[b,(o,n)] and (non-final iters) srep[(b,il),(o,n)]
    sym = small.tile([80, ON], bf16, tag="sym")
    nc.vector.tensor_mul(sym[:], pss[:], dmask_s[:])
    s3p = psX.tile([BC, ON], f32, tag="s3p")
    nc.tensor.matmul(s3p[:], selb_s[:], sym[:], start=True, stop=True)
    s3 = small.tile([BC, ON], f32, tag="s3")
    nc.scalar.copy(s3[:], s3p[:])

    # squash scale sc[b,o] = nsq * rsqrt(nsq*(1+nsq)^2), fp32
    sq = small.tile([BC, ON], bf16, tag="sq")
    nc.scalar.square(sq[:], s3[:])
    nsq = small.tile([BC, O], f32, tag="nsq")
    nc.vector.tensor_reduce(
        nsq[:], sq[:].rearrange("b (o n) -> b o n", n=N), axis=AX.X, op=OP.add
    )
    d1 = small.tile([BC, O], f32, tag="d1")
    nc.scalar.activation(d1[:], nsq[:], AF.Square, bias=1.0)  # (1+nsq)^2
    dd = small.tile([BC, O], f32, tag="dd")
    nc.vector.tensor_mul(dd[:], d1[:], nsq[:])
    # rsqrt(dd): bit hack + 2 Newton steps (DVE + ACT, no new ACT tables)
    yy = small.tile([BC, O], f32, tag="yy")
    nc.vector.tensor_scalar(
        yy[:].bitcast(i32), dd[:].bitcast(i32), 1, None,
        op0=OP.logical_shift_right,
    )
    nc.vector.tensor_scalar(
        yy[:].bitcast(i32), yy[:].bitcast(i32), -1, RSQRT_MAGIC,
        op0=OP.mult, op1=OP.add,
    )
    for _ in range(1):
        y2 = small.tile([BC, O], f32, tag="y2")
        nc.scalar.square(y2[:], yy[:])
        t2 = small.tile([BC, O], f32, tag="t2")
        nc.vector.tensor_mul(t2[:], y2[:], dd[:])
        u2 = small.tile([BC, O], f32, tag="u2")
        nc.scalar.activation(u2[:], t2[:], AF.Copy, bias=1.5, scale=-0.5)
        yn = small.tile([BC, O], f32, tag="yn")
        nc.vector.tensor_mul(yn[:], yy[:], u2[:])
        yy = yn
    sc = small.tile([BC, O], f32, tag="sc")
    nc.vector.tensor_mul(sc[:], nsq[:], yy[:])

    if last:
        v3f = small.tile([BC, ON], f32, tag="v3f", bufs=1)
        nc.vector.tensor_mul(
            v3f[:].rearrange("b (o n) -> b o n", n=N),
            s3[:].rearrange("b (o n) -> b o n", n=N),
            sc[:].unsqueeze(2).broadcast_to([BC, O, N]),
        )
        nc.sync.dma_start(
            out=out_d[c * BC:(c + 1) * BC, :, :],
            in_=v3f[:].rearrange("b (o n) -> b o n", n=N),
        )
        return

    # replicated v: srep[(b,il),(o,n)] * screp[(b,il),o]
    srep = psE.tile([128, ON], f32, tag="srep")
    nc.tensor.matmul(srep[:], selr_s[:], sym[:], start=True, stop=True)
    sreps = small.tile([128, ON], bf16, tag="sreps")
    nc.scalar.copy(sreps[:], srep[:])
    nc.vector.tensor_copy(scb[0:BC, :], sc[:])
    screp = small.tile([128, O], bf16, tag="screp")
    for q in range(4):
        nc.vector.stream_shuffle(
            screp[q * 32:(q + 1) * 32, :],
            scb[:],
            [2 * q + (j // 16) for j in range(32)],
        )
    vrep = small.tile([128, ON], bf16, tag="vrep")
    nc.vector.tensor_mul(
        vrep[:].rearrange("p (o n) -> p o n", n=N),
        sreps[:].rearrange("p (o n) -> p o n", n=N),
        screp[:].unsqueeze(2).broadcast_to([128, O, N]),
    )

    # agreement a[(b,il), ib, o] = sum_n U*vrep, bf16 add-tree per half
    H = IB // 2
    a2 = None
    if it > 0:
        a2 = small.tile([128, IB, O], bf16, tag="a2")
    for h in range(2):
        ph = tree.tile([128, H, O, N], bf16, tag="ph")
        nc.vector.tensor_mul(
            ph[:],
            U[:, h * H:(h + 1) * H, :].rearrange("p i (o n) -> p i o n", n=N),
            vrep[:]
            .rearrange("p (o n) -> p o n", n=N)
            .unsqueeze(1)
            .broadcast_to([128, H, O, N]),
        )
        t8 = tree.tile([128, H, O, 8], bf16, tag="t8")
        nc.vector.tensor_add(t8[:], ph[:, :, :, 0:8], ph[:, :, :, 8:16])
        t4 = tree.tile([128, H, O, 4], bf16, tag="t4")
        nc.vector.tensor_add(t4[:], t8[:, :, :, 0:4], t8[:, :, :, 4:8])
        t2t = tree.tile([128, H, O, 2], bf16, tag="t2t")
        nc.vector.tensor_add(t2t[:], t4[:, :, :, 0:2], t4[:, :, :, 2:4])
        dsta = (bb if it == 0 else a2)[:, h * H:(h + 1) * H, :]
        nc.vector.tensor_add(dsta, t2t[:, :, :, 0], t2t[:, :, :, 1])
    if it == 0:
        bbcur = bb
    else:
        bb2 = small.tile([128, IB, O], bf16, tag="bb2")
        nc.vector.tensor_add(bb2[:], bb[:], a2[:])
        if it < n_routing - 2:
            nc.vector.tensor_copy(bb[:], bb2[:])
        bbcur = bb2

    # softmax over o -> c2n [128, O, IB] bf16, scatter diag into CBD
    c2 = small.tile([128, O, IB], bf16, tag="c2")
    nc.scalar.activation(c2[:].transpose([0, 2, 1]), bbcur[:], AF.Exp)
    ssum = small.tile([128, IB], f32, tag="ssum")
    nc.vector.tensor_reduce(
        ssum[:], c2[:].transpose([0, 2, 1]), axis=AX.X, op=OP.add
    )
    rs = small.tile([128, IB], bf16, tag="rs")
    with nc.allow_low_precision(reason="softmax weights tolerate bf16"):
        nc.vector.reciprocal(rs[:], ssum[:])
    c2n = small.tile([128, O, IB], bf16, tag="c2n")
    nc.vector.tensor_mul(
        c2n[:], c2[:], rs[:].unsqueeze(1).broadcast_to([128, O, IB])
    )
    for b in range(BC):
        nc.sync.dma_start(
            out=cbd[b * 16:(b + 1) * 16, b:80:8, :],
            in_=c2n[b * 16:(b + 1) * 16, :, :],
        )


_CACHE = {}


def _get(n_routing: int):
    if n_routing not in _CACHE:
        _CACHE[n_routing] = _build(n_routing)
    return _CACHE[n_routing]


def _bf16(a):
    import ml_dtypes

    return np.asarray(a, dtype=ml_dtypes.bfloat16)


def _prep_host(inputs: np.ndarray, W: np.ndarray):
    x = np.ascontiguousarray(np.asarray(inputs, dtype=np.float32))
    W = np.asarray(W, dtype=np.float32)
    # w2[(il,v), ib, (o,n)] = W[ib*16+il, o, v, n]
    w2 = np.ascontiguousarray(
        W.reshape(IB, 16, O, V, N).transpose(1, 3, 0, 2, 4).reshape(128, IB, ON)
    )
    # e2[(b,il), (o,b')] = 0.1 * (b == b')   (uniform softmax weights)
    e2 = np.zeros((128, 80), dtype=np.float32)
    for b in range(BC):
        e2[b * 16:(b + 1) * 16, np.arange(O) * 8 + b] = 0.1
    # dmask[(o,b), (o',n)] = (o == o')
    dmask = np.zeros((80, ON), dtype=np.float32)
    for o in range(O):
        dmask[o * 8:(o + 1) * 8, o * N:(o + 1) * N] = 1.0
    # selb[(o,b), b'] = (b == b') : s3[b',(o,n)] = sum_o sym[(o,b'),(o,n)]
    selb = np.zeros((80, BC), dtype=np.float32)
    for o in range(O):
        for b in range(BC):
            selb[o * 8 + b, b] = 1.0
    # selr[(o,b), (b',il)] = (b == b') : replicate to all 128 partitions
    selr = np.zeros((80, 128), dtype=np.float32)
    for o in range(O):
        for b in range(BC):
            selr[o * 8 + b, b * 16:(b + 1) * 16] = 1.0
    return x, _bf16(w2), _bf16(e2), _bf16(dmask), _bf16(selb), _bf16(selr)


def _make_in_maps(inputs, W):
    x, w2, e2, dmask, selb, selr = _prep_host(inputs, W)
    in_maps = []
    for core in range(NCORES):
        xc = x[core * BLOC:(core + 1) * BLOC]              # [32, 1152, 8]
        # xbdh[c, il*8+v, ib, b*16+il] = xc[c*BC+b, ib*16+il, v]
        xr = xc.reshape(NCHUNK, BC, IB, 16, V)
        xbdh = np.zeros((NCHUNK, 128, IB, 128), dtype=np.float32)
        for il in range(16):
            xbdh[:, il * 8:(il + 1) * 8, :, il::16] = xr[:, :, :, il, :].transpose(
                0, 3, 2, 1
            )
        in_maps.append(
            {
                "xbdh": _bf16(xbdh),
                "w2": w2,
                "e2": e2,
                "dmask": dmask,
                "selb": selb,
                "selr": selr,
            }
        )
    return in_maps


def kernel(inputs, W, n_routing):
    from concourse.bass_utils import run_bass_kernel_spmd

    n_routing = int(n_routing)
    nc = _get(n_routing)
    in_maps = _make_in_maps(inputs, W)
    res = run_bass_kernel_spmd(nc, in_maps, core_ids=list(range(NCORES)))
    outs = [res.results[i]["out"] for i in range(NCORES)]
    return np.concatenate(outs, axis=0).astype(np.float32)


# revision 25
# speedup vs baseline: 1.0952x; 1.0952x over previous
"""CapsuleLayer (dynamic routing) Trainium2 Bass kernel — v2.

Math (per example b):
  u_hat[b,i,o,n] = sum_v x[b,i,v] * W[i,o,v,n]        I=1152, O=10, V=8, N=16
  b_logits = 0; repeat n_routing times:
    c = softmax_o(b_logits); s = sum_i c*u_hat; out = squash(s)
    if not last: b_logits += sum_n u_hat*out

Distribution: batch B=256 sharded over 8 cores (32 each). W replicated.

Per-core layout (chunk = 8 examples, 4 chunks), i = ib*16 + il:
  K partitions k = il*8+v   (contraction rows of the u_hat matmul)
  M partitions p = b*16+il  (rows of u_hat / routing state)
  U[c] [128, 72, 160] bf16  u_hat,  U[(b,il), ib, (o,n)]
  XBD  [128, 9, 128] bf16 slabs of block-diag x: XBD[(il,v),ib,(b,il')]
  CBD[c] [128, 80, 72] bf16 block-diag c: CBD[(b,il),(o,b'),ib]
  w2   [128, 72, 160] bf16  W2[(il,v), ib, (o,n)] = W[ib*16+il, o, v, n]
  u_hat matmul (per ib): psum[(b,il'),(o,n)] = XBD[:,ib,:].T @ w2[:,ib,:]
  s matmul (per iter): psum[(o,b),(o',n)] += CBD[:,:,ib].T @ U[:,ib,:]

v2 structure (vs v1 pairs): all 4 chunks in flight; iters outer, chunks
inner, with phase-1 of chunks 2/3 interleaved after routing of chunks 0/1
starts, so PE runs continuously and DVE agreement pipelines across chunks.
Diagonal s-extraction is mask-multiply + tiny selection matmuls (replacing
10 small DMAs per chunk-iter); a second selection matmul replicates s to
all 128 partitions for the agreement product. Small DMAs (cbd scatter,
output) ride the gpsimd queue (cheap issue). psum->U casts round-robin
over DVE/ACT/GPSIMD. Routing-side tensors (b-logits, softmax, agreement
tree) are bf16 (DVE 2x); squash runs fp32 with ACT doing the pointwise
chain and a DVE-only rsqrt (bit hack + 2 Newton steps).
"""

import os
import sys

import numpy as np

_TRN_REPO = "/opt/trn_rl_repo"
if _TRN_REPO not in sys.path:
    sys.path.insert(0, _TRN_REPO)

EPS = 1e-10
B, I, V, O, N = 256, 1152, 8, 10, 16
NCORES = 8
BLOC = B // NCORES          # 32 examples per core
BC = 8                      # examples per chunk
NCHUNK = BLOC // BC         # 4
IB = I // 16                # 72 i-blocks
ON = O * N                  # 160
NSLAB = 8                   # phase-1 xbd slabs per chunk
SLAB = IB // NSLAB          # 9 ibs per slab
RSQRT_MAGIC = 0x5F3759DF


def _build(n_routing: int):
    import concourse.bacc as bacc
    import concourse.tile as tile
    from concourse import mybir

    nc = bacc.Bacc("TRN2", target_bir_lowering=False, debug=False)
    f32 = mybir.dt.float32
    bf16 = mybir.dt.bfloat16

    xbdh = nc.dram_tensor(
        "xbdh", [NCHUNK, 128, IB, 128], bf16, kind="ExternalInput"
    )
    w2 = nc.dram_tensor("w2", [128, IB, ON], bf16, kind="ExternalInput")
    e2 = nc.dram_tensor("e2", [128, 80], bf16, kind="ExternalInput")
    dmask = nc.dram_tensor("dmask", [80, ON], bf16, kind="ExternalInput")
    selb = nc.dram_tensor("selb", [80, BC], bf16, kind="ExternalInput")
    selr = nc.dram_tensor("selr", [80, 128], bf16, kind="ExternalInput")
    out_d = nc.dram_tensor("out", [BLOC, O, N], f32, kind="ExternalOutput")

    with tile.TileContext(nc) as tc:
        with (
            tc.tile_pool(name="state", bufs=1) as state,
            tc.tile_pool(name="p1", bufs=1) as p1,
            tc.tile_pool(name="small", bufs=2) as small,
            tc.tile_pool(name="tree", bufs=1) as tree,
            tc.tile_pool(name="psA", bufs=3, space="PSUM") as psA,
            tc.tile_pool(name="psS", bufs=3, space="PSUM") as psS,
            tc.tile_pool(name="psE", bufs=1, space="PSUM") as psE,
            tc.tile_pool(name="psX", bufs=1, space="PSUM") as psX,
        ):
            Us = [
                state.tile([128, IB, ON], bf16, tag=f"U{j}", name=f"U{j}")
                for j in range(NCHUNK)
            ]
            cbds = [
                state.tile([128, 80, IB], bf16, tag=f"cbd{j}", name=f"cbd{j}")
                for j in range(NCHUNK)
            ] if n_routing > 1 else []
            bbs = [
                state.tile([128, IB, O], bf16, tag=f"bb{j}", name=f"bb{j}")
                for j in range(NCHUNK)
            ]
            e2s = state.tile([128, 80], bf16)
            nc.scalar.dma_start(out=e2s[:], in_=e2[:])
            dmask_s = state.tile([80, ON], bf16)
            nc.scalar.dma_start(out=dmask_s[:], in_=dmask[:])
            selb_s = state.tile([80, BC], bf16)
            nc.scalar.dma_start(out=selb_s[:], in_=selb[:])
            selr_s = state.tile([80, 128], bf16)
            nc.scalar.dma_start(out=selr_s[:], in_=selr[:])
            w2s = state.tile([128, IB, ON], bf16)
            for sl in range(NSLAB):
                nc.scalar.dma_start(
                    out=w2s[:, sl * SLAB:(sl + 1) * SLAB, :],
                    in_=w2[:, sl * SLAB:(sl + 1) * SLAB, :],
                )
            # zero block-diag c while phase 1 runs (gpsimd is idle)
            for cb in cbds:
                nc.gpsimd.memset(cb[:], 0.0)
            scb = state.tile([32, O], bf16)
            nc.gpsimd.memset(scb[:], 0.0)

            def phase1(c):
                for sl in range(NSLAB):
                    xbd = p1.tile([128, SLAB, 128], bf16, tag="xbd", bufs=2)
                    nc.sync.dma_start(
                        out=xbd[:], in_=xbdh[c, :, sl * SLAB:(sl + 1) * SLAB, :]
                    )
                    for g in range(SLAB // 3):
                        ps = psA.tile([128, 3, ON], f32, tag="psA")
                        for j in range(3):
                            ib = g * 3 + j
                            nc.tensor.matmul(
                                ps[:, j, :],
                                xbd[:, ib, :],
                                w2s[:, sl * SLAB + ib, :],
                                start=True,
                                stop=True,
                            )
                        dst = Us[c][:, sl * SLAB + g * 3:sl * SLAB + (g + 1) * 3, :]
                        if (sl * 3 + g) % 3 == 0:
                            nc.vector.tensor_copy(dst, ps[:])
                        else:
                            nc.scalar.copy(dst, ps[:])

            f32 = mybir.dt.float32

            def s_part(c, it):
                pss = psS.tile([80, ON], f32, tag="psS")
                for ib in range(IB):
                    lhsT = e2s[:] if it == 0 else cbds[c][:, :, ib]
                    nc.tensor.matmul(
                        pss[:], lhsT, Us[c][:, ib, :],
                        start=(ib == 0), stop=(ib == IB - 1),
                    )
                return pss

            def rest(c, it, pss):
                _routing_rest(
                    nc, tc, mybir, small, tree, psE, psX, pss,
                    Us[c], cbds[c] if cbds else None, bbs[c], scb,
                    dmask_s, selb_s, selr_s, out_d, c, it, n_routing,
                )

            # interleave phase 1 with the first routing iteration, and emit
            # each chunk's extraction after the next chunk's s-matmuls so PE
            # never waits on the (DVE) mask step
            phase1(0)
            phase1(1)
            ps0 = s_part(0, 0)
            phase1(2)
            ps1 = s_part(1, 0)
            rest(0, 0, ps0)
            phase1(3)
            ps2 = s_part(2, 0)
            rest(1, 0, ps1)
            ps3 = s_part(3, 0)
            rest(2, 0, ps2)
            rest(3, 0, ps3)
            for it in range(1, n_routing):
                ps0 = s_part(0, it)
                ps1 = s_part(1, it)
                rest(0, it, ps0)
                ps2 = s_part(2, it)
                rest(1, it, ps1)
                ps3 = s_part(3, it)
                rest(2, it, ps2)
                rest(3, it, ps3)

    nc.compile()
    return nc


def _routing_rest(nc, tc, mybir, small, tree, psE, psX, pss, U, cbd, bb,
                  scb, dmask_s, selb_s, selr_s, out_d, c, it, n_routing):
    f32 = mybir.dt.float32
    bf16 = mybir.dt.bfloat16
    i32 = mybir.dt.int32
    AX = mybir.AxisListType
    OP = mybir.AluOpType
    AF = mybir.ActivationFunctionType
    last = it == n_routing - 1

    # mask off-diagonal o-blocks, then sum partition o-groups with tiny
    # matmuls: s3p[b,(o,n)] and (non-final iters) srep[(b,il),(o,n)]
    sym = small.tile([80, ON], bf16, tag="sym")
    nc.vector.tensor_mul(sym[:], pss[:], dmask_s[:])
    s3p = psX.tile([BC, ON], f32, tag="s3p")
    nc.tensor.matmul(s3p[:], selb_s[:], sym[:], start=True, stop=True)
    s3 = small.tile([BC, ON], f32, tag="s3")
    nc.scalar.copy(s3[:], s3p[:])

    # squash scale sc[b,o] = nsq * rsqrt(nsq*(1+nsq)^2), fp32
    sq = small.tile([BC, ON], bf16, tag="sq")
    nc.scalar.square(sq[:], s3[:])
    nsq = small.tile([BC, O], f32, tag="nsq")
    nc.vector.tensor_reduce(
        nsq[:], sq[:].rearrange("b (o n) -> b o n", n=N), axis=AX.X, op=OP.add
    )
    d1 = small.tile([BC, O], f32, tag="d1")
    nc.scalar.activation(d1[:], nsq[:], AF.Square, bias=1.0)  # (1+nsq)^2
    dd = small.tile([BC, O], f32, tag="dd")
    nc.vector.tensor_mul(dd[:], d1[:], nsq[:])
    # rsqrt(dd): bit hack + 2 Newton steps (DVE + ACT, no new ACT tables)
    yy = small.tile([BC, O], f32, tag="yy")
    nc.vector.tensor_scalar(
        yy[:].bitcast(i32), dd[:].bitcast(i32), 1, None,
        op0=OP.logical_shift_right,
    )
    nc.vector.tensor_scalar(
        yy[:].bitcast(i32), yy[:].bitcast(i32), -1, RSQRT_MAGIC,
        op0=OP.mult, op1=OP.add,
    )
    for _ in range(1):
        y2 = small.tile([BC, O], f32, tag="y2")
        nc.scalar.square(y2[:], yy[:])
        t2 = small.tile([BC, O], f32, tag="t2")
        nc.vector.tensor_mul(t2[:], y2[:], dd[:])
        u2 = small.tile([BC, O], f32, tag="u2")
        nc.scalar.activation(u2[:], t2[:], AF.Copy, bias=1.5, scale=-0.5)
        yn = small.tile([BC, O], f32, tag="yn")
        nc.vector.tensor_mul(yn[:], yy[:], u2[:])
        yy = yn
    sc = small.tile([BC, O], f32, tag="sc")
    nc.vector.tensor_mul(sc[:], nsq[:], yy[:])

    if last:
        v3f = small.tile([BC, ON], f32, tag="v3f", bufs=1)
        nc.vector.tensor_mul(
            v3f[:].rearrange("b (o n) -> b o n", n=N),
            s3[:].rearrange("b (o n) -> b o n", n=N),
            sc[:].unsqueeze(2).broadcast_to([BC, O, N]),
        )
        nc.sync.dma_start(
            out=out_d[c * BC:(c + 1) * BC, :, :],
            in_=v3f[:].rearrange("b (o n) -> b o n", n=N),
        )
        return

    # replicated v: srep[(b,il),(o,n)] * screp[(b,il),o]
    srep = psE.tile([128, ON], f32, tag="srep")
    nc.tensor.matmul(srep[:], selr_s[:], sym[:], start=True, stop=True)
    sreps = small.tile([128, ON], bf16, tag="sreps")
    nc.scalar.copy(sreps[:], srep[:])
    nc.vector.tensor_copy(scb[0:BC, :], sc[:])
    screp = small.tile([128, O], bf16, tag="screp")
    for q in range(4):
        nc.vector.stream_shuffle(
            screp[q * 32:(q + 1) * 32, :],
            scb[:],
            [2 * q + (j // 16) for j in range(32)],
        )
    vrep = small.tile([128, ON], bf16, tag="vrep")
    nc.vector.tensor_mul(
        vrep[:].rearrange("p (o n) -> p o n", n=N),
        sreps[:].rearrange("p (o n) -> p o n", n=N),
        screp[:].unsqueeze(2).broadcast_to([128, O, N]),
    )

    # agreement a[(b,il), ib, o] = sum_n U*vrep, bf16 add-tree per half
    H = IB // 2
    a2 = None
    if it > 0:
        a2 = small.tile([128, IB, O], bf16, tag="a2")
    for h in range(2):
        ph = tree.tile([128, H, O, N], bf16, tag="ph")
        nc.vector.tensor_mul(
            ph[:],
            U[:, h * H:(h + 1) * H, :].rearrange("p i (o n) -> p i o n", n=N),
            vrep[:]
            .rearrange("p (o n) -> p o n", n=N)
            .unsqueeze(1)
            .broadcast_to([128, H, O, N]),
        )
        t8 = tree.tile([128, H, O, 8], bf16, tag="t8")
        nc.vector.tensor_add(t8[:], ph[:, :, :, 0:8], ph[:, :, :, 8:16])
        t4 = tree.tile([128, H, O, 4], bf16, tag="t4")
        nc.vector.tensor_add(t4[:], t8[:, :, :, 0:4], t8[:, :, :, 4:8])
        t2t = tree.tile([128, H, O, 2], bf16, tag="t2t")
        nc.vector.tensor_add(t2t[:], t4[:, :, :, 0:2], t4[:, :, :, 2:4])
        dsta = (bb if it == 0 else a2)[:, h * H:(h + 1) * H, :]
        nc.vector.tensor_add(dsta, t2t[:, :, :, 0], t2t[:, :, :, 1])
    if it == 0:
        bbcur = bb
    else:
        bb2 = small.tile([128, IB, O], bf16, tag="bb2")
        nc.vector.tensor_add(bb2[:], bb[:], a2[:])
        if it < n_routing - 2:
            nc.vector.tensor_copy(bb[:], bb2[:])
        bbcur = bb2

    # softmax over o -> c2n [128, O, IB] bf16, scatter diag into CBD
    c2 = small.tile([128, O, IB], bf16, tag="c2")
    nc.scalar.activation(c2[:].transpose([0, 2, 1]), bbcur[:], AF.Exp)
    ssum = small.tile([128, IB], f32, tag="ssum")
    nc.vector.tensor_reduce(
        ssum[:], c2[:].transpose([0, 2, 1]), axis=AX.X, op=OP.add
    )
    rs = small.tile([128, IB], bf16, tag="rs")
    with nc.allow_low_precision(reason="softmax weights tolerate bf16"):
        nc.vector.reciprocal(rs[:], ssum[:])
    c2n = small.tile([128, O, IB], bf16, tag="c2n")
    nc.vector.tensor_mul(
        c2n[:], c2[:], rs[:].unsqueeze(1).broadcast_to([128, O, IB])
    )
    for b in range(BC):
        nc.sync.dma_start(
            out=cbd[b * 16:(b + 1) * 16, b:80:8, :],
            in_=c2n[b * 16:(b + 1) * 16, :, :],
        )


_CACHE = {}


def _get(n_routing: int):
    if n_routing not in _CACHE:
        _CACHE[n_routing] = _build(n_routing)
    return _CACHE[n_routing]


def _bf16(a):
    import ml_dtypes

    return np.asarray(a, dtype=ml_dtypes.bfloat16)


def _prep_host(inputs: np.ndarray, W: np.ndarray):
    x = np.ascontiguousarray(np.asarray(inputs, dtype=np.float32))
    W = np.asarray(W, dtype=np.float32)
    # w2[(il,v), ib, (o,n)] = W[ib*16+il, o, v, n]
    w2 = np.ascontiguousarray(
        W.reshape(IB, 16, O, V, N).transpose(1, 3, 0, 2, 4).reshape(128, IB, ON)
    )
    # e2[(b,il), (o,b')] = 0.1 * (b == b')   (uniform softmax weights)
    e2 = np.zeros((128, 80), dtype=np.float32)
    for b in range(BC):
        e2[b * 16:(b + 1) * 16, np.arange(O) * 8 + b] = 0.1
    # dmask[(o,b), (o',n)] = (o == o')
    dmask = np.zeros((80, ON), dtype=np.float32)
    for o in range(O):
        dmask[o * 8:(o + 1) * 8, o * N:(o + 1) * N] = 1.0
    # selb[(o,b), b'] = (b == b') : s3[b',(o,n)] = sum_o sym[(o,b'),(o,n)]
    selb = np.zeros((80, BC), dtype=np.float32)
    for o in range(O):
        for b in range(BC):
            selb[o * 8 + b, b] = 1.0
    # selr[(o,b), (b',il)] = (b == b') : replicate to all 128 partitions
    selr = np.zeros((80, 128), dtype=np.float32)
    for o in range(O):
        for b in range(BC):
            selr[o * 8 + b, b * 16:(b + 1) * 16] = 1.0
    return x, _bf16(w2), _bf16(e2), _bf16(dmask), _bf16(selb), _bf16(selr)


def _make_in_maps(inputs, W):
    x, w2, e2, dmask, selb, selr = _prep_host(inputs, W)
    in_maps = []
    for core in range(NCORES):
        xc = x[core * BLOC:(core + 1) * BLOC]              # [32, 1152, 8]
        # xbdh[c, il*8+v, ib, b*16+il] = xc[c*BC+b, ib*16+il, v]
        xr = xc.reshape(NCHUNK, BC, IB, 16, V)
        xbdh = np.zeros((NCHUNK, 128, IB, 128), dtype=np.float32)
        for il in range(16):
            xbdh[:, il * 8:(il + 1) * 8, :, il::16] = xr[:, :, :, il, :].transpose(
                0, 3, 2, 1
            )
        in_maps.append(
            {
                "xbdh": _bf16(xbdh),
                "w2": w2,
                "e2": e2,
                "dmask": dmask,
                "selb": selb,
                "selr": selr,
            }
        )
    return in_maps


def kernel(inputs, W, n_routing):
    from concourse.bass_utils import run_bass_kernel_spmd

    n_routing = int(n_routing)
    nc = _get(n_routing)
    in_maps = _make_in_maps(inputs, W)
    res = run_bass_kernel_spmd(nc, in_maps, core_ids=list(range(NCORES)))
    outs = [res.results[i]["out"] for i in range(NCORES)]
    return np.concatenate(outs, axis=0).astype(np.float32)
